# revision 25
# baseline (speedup 1.0000x reference)
"""Trainium2 Bass kernel for nn_BackgroundStd2D.

Computes, per (batch, channel): the unbiased std of bf over the pixels where
aspp_mask <= 0.5, clamped below by min_std + 1e-6.

Sharding: pure data parallel over the 1024 (batch, channel) rows of
bf.reshape(B*C, H*W); each of the 8 cores handles 128 rows (one batch's
half of channels) plus that batch's mask.

Per-core algorithm (rows on partitions, pixels on the free axis):
  keep128[p, f] = (mask[p*512+f] <= 0.5)                      [128, 512]
  PE broadcasts keep rows across partitions via ones-matmul into PSUM.
  DVE tensor_tensor_reduce: bfk = bf * keep, s_part = sum(bfk) (fused).
  ACT activation(Square, accum_out): q_part = sum(bfk^2).
  n via free-axis reduce of keep128 + gpsimd partition_all_reduce.
  Final [128,1] math: std = sqrt((q - s^2/n) / (n-1)) (+ Newton refines),
  out = max(std, min_std + 1e-6).
"""

import sys

sys.path.insert(0, "/opt/trn_rl_repo")

import numpy as np

import concourse.bass as bass
import concourse.tile as tile
from concourse import bacc, mybir
from concourse.bass_utils import run_bass_kernel_spmd

P = 128
N_CORES = 8
MIN_STD_VAL = 1e-05

F32 = mybir.dt.float32
ALU = mybir.AluOpType
ACTF = mybir.ActivationFunctionType


def build_bass(
    npix: int, dma_chunk: int = 4096, ttr_chunk: int = 2048, iters: int = 1
) -> bass.Bass:
    assert npix % dma_chunk == 0 and dma_chunk % ttr_chunk == 0
    assert ttr_chunk % 512 == 0
    n_blk = npix // 512  # 512-pixel blocks; one keep row per block
    assert n_blk <= P
    n_dma = npix // dma_chunk
    n_ttr = npix // ttr_chunk

    nc = bacc.Bacc("TRN2", target_bir_lowering=False, debug=False)

    bf_d = nc.dram_tensor("bf", [P, npix], F32, kind="ExternalInput").ap()
    mask_d = nc.dram_tensor("mask", [n_blk, 512], F32, kind="ExternalInput").ap()
    mins_d = nc.dram_tensor("min_std", [P, 1], F32, kind="ExternalInput").ap()
    out_d = nc.dram_tensor("out", [P, 1], F32, kind="ExternalOutput").ap()
    keep_scratch = nc.dram_tensor("keep_scratch", [npix], F32).ap()
    cnt_scratch = nc.dram_tensor("cnt_scratch", [P], F32).ap()
    n_scratch = nc.dram_tensor("n_scratch", [1], F32).ap()
    nsel = ttr_chunk // 512
    sel_d = nc.dram_tensor("sels", [nsel, nsel, P], F32, kind="ExternalInput").ap()

    with tile.TileContext(nc) as tc:
        with (
            tc.tile_pool(name="singles", bufs=1) as singles,
            tc.tile_pool(name="bfp", bufs=3) as bf_pool,
            tc.tile_pool(name="bfkp", bufs=3) as bfk_pool,
            tc.tile_pool(name="kps", bufs=2, space="PSUM") as kp_pool,
            tc.tile_pool(name="fin", bufs=2) as fin,
        ):
            # One-hot row selectors: sel[k].T @ keep_r[:, J, :] broadcasts
            # keep row k across all 128 output partitions.
            sel_t = singles.tile([nsel, nsel, P], F32)
            nc.sync.dma_start(out=sel_t, in_=sel_d)
            sels = [sel_t[:, k, :] for k in range(nsel)]

            mask128 = singles.tile([n_blk, 512], F32)
            nc.sync.dma_start(out=mask128, in_=mask_d)
            keep128 = singles.tile([n_blk, 512], F32)
            nc.vector.tensor_scalar(
                out=keep128, in0=mask128, scalar1=0.5, scalar2=None, op0=ALU.is_le
            )
            # Bounce through DRAM to land keep in [4, n_ttr, 512] layout:
            # partition a holds pixel blocks {4m + a}.
            nc.sync.dma_start(out=keep_scratch, in_=keep128)
            keep_r = singles.tile([nsel, n_ttr, 512], F32)
            nc.sync.dma_start(
                out=keep_r,
                in_=keep_scratch.rearrange("(m a f) -> a m f", a=nsel, f=512),
            )

            # n = sum(keep): free-axis reduce, then fold the 128 partition
            # partials onto one partition via a DRAM bounce, reduce, and
            # broadcast the scalar back to all partitions.
            cnt = singles.tile([P, 1], F32)
            nc.vector.memset(cnt, 0.0)
            nc.vector.reduce_sum(
                out=cnt[0:n_blk, :], in_=keep128, axis=mybir.AxisListType.X
            )
            nc.sync.dma_start(out=cnt_scratch, in_=cnt)
            cnt_row = singles.tile([1, P], F32)
            nc.sync.dma_start(out=cnt_row, in_=cnt_scratch)
            n_scalar = singles.tile([1, 1], F32)
            nc.vector.reduce_sum(out=n_scalar, in_=cnt_row, axis=mybir.AxisListType.X)
            nc.sync.dma_start(out=n_scratch, in_=n_scalar)
            n_b = singles.tile([P, 1], F32)
            nc.sync.dma_start(out=n_b, in_=n_scratch.to_broadcast([P, 1]))

            minstd_sb = singles.tile([P, 1], F32)
            nc.sync.dma_start(out=minstd_sb, in_=mins_d)

            s_parts = singles.tile([P, n_ttr], F32)
            q_parts = singles.tile([P, n_dma], F32)

            for _it in range(iters):
              for c in range(n_dma):
                bf_t = bf_pool.tile([P, dma_chunk], F32)
                nc.sync.dma_start(out=bf_t, in_=bf_d[:, c * dma_chunk : (c + 1) * dma_chunk])
                bfk_t = bfk_pool.tile([P, dma_chunk], F32)
                for h in range(dma_chunk // ttr_chunk):
                    j = c * (dma_chunk // ttr_chunk) + h
                    kp = kp_pool.tile([P, ttr_chunk], F32)
                    for k in range(ttr_chunk // 512):
                        nc.tensor.matmul(
                            kp[:, 512 * k : 512 * (k + 1)],
                            sels[k],
                            keep_r[:, j, :],
                            start=True,
                            stop=True,
                        )
                    nc.vector.scalar_tensor_tensor(
                        out=bfk_t[:, h * ttr_chunk : (h + 1) * ttr_chunk],
                        in0=bf_t[:, h * ttr_chunk : (h + 1) * ttr_chunk],
                        scalar=1.0,
                        in1=kp,
                        op0=ALU.mult,
                        op1=ALU.mult,
                        accum_out=s_parts[:, j : j + 1],
                    )
                nc.scalar.activation(
                    out=bfk_t,
                    in_=bfk_t,
                    func=ACTF.Square,
                    accum_out=q_parts[:, c : c + 1],
                )

            s = fin.tile([P, 1], F32)
            nc.vector.reduce_sum(out=s, in_=s_parts, axis=mybir.AxisListType.X)
            q = fin.tile([P, 1], F32)
            nc.vector.reduce_sum(out=q, in_=q_parts, axis=mybir.AxisListType.X)

            inv_n = fin.tile([P, 1], F32)
            nc.vector.reciprocal(inv_n, n_b)
            mean = fin.tile([P, 1], F32)
            nc.vector.tensor_mul(mean, s, inv_n)
            s2n = fin.tile([P, 1], F32)
            nc.vector.tensor_mul(s2n, mean, s)
            num = fin.tile([P, 1], F32)
            nc.vector.tensor_sub(num, q, s2n)
            nm1 = fin.tile([P, 1], F32)
            nc.vector.tensor_scalar_add(nm1, n_b, -1.0)
            inv_nm1 = fin.tile([P, 1], F32)
            nc.vector.reciprocal(inv_nm1, nm1)
            var = fin.tile([P, 1], F32)
            nc.vector.tensor_mul(var, num, inv_nm1)

            std = fin.tile([P, 1], F32)
            nc.scalar.sqrt(std, var)
            # ACT sqrt has a loose ULP budget; two Newton steps pin it to f32.
            for it in range(2):
                r = fin.tile([P, 1], F32, name=f"r{it}")
                nc.vector.reciprocal(r, std)
                t = fin.tile([P, 1], F32, name=f"t{it}")
                nc.vector.tensor_mul(t, var, r)
                u = fin.tile([P, 1], F32, name=f"u{it}")
                nc.vector.tensor_add(u, std, t)
                std = fin.tile([P, 1], F32, name=f"std{it}")
                nc.vector.tensor_scalar_mul(std, u, 0.5)

            lower = fin.tile([P, 1], F32)
            nc.vector.tensor_scalar_add(lower, minstd_sb, MIN_STD_VAL / 10.0)
            outv = fin.tile([P, 1], F32)
            nc.vector.tensor_max(outv, std, lower)
            nc.sync.dma_start(out=out_d, in_=outv)

    nc.compile()
    return nc


_NC_CACHE: dict[tuple, bass.Bass] = {}


def _get_nc(npix: int, **kwargs) -> bass.Bass:
    key = (npix, tuple(sorted(kwargs.items())))
    if key not in _NC_CACHE:
        _NC_CACHE[key] = build_bass(npix, **kwargs)
    return _NC_CACHE[key]


def make_in_maps(bf: np.ndarray, aspp_mask: np.ndarray, min_std: np.ndarray):
    B, C, H, W = bf.shape
    npix = H * W
    rows = bf.reshape(B * C, npix)
    rows_per_core = (B * C) // N_CORES
    cores_per_batch = C // rows_per_core
    mask_flat = np.ascontiguousarray(aspp_mask.reshape(B, npix))
    minstd_flat = np.ascontiguousarray(min_std.reshape(C))
    sels = make_sels()
    in_maps = []
    for k in range(N_CORES):
        b = k // cores_per_batch
        c0 = (k % cores_per_batch) * rows_per_core
        in_maps.append(
            {
                "bf": np.ascontiguousarray(rows[k * rows_per_core : (k + 1) * rows_per_core]),
                "mask": mask_flat[b].reshape(npix // 512, 512),
                "min_std": minstd_flat[c0 : c0 + rows_per_core].reshape(P, 1),
                "sels": sels,
            }
        )
    return in_maps


def make_sels(nsel: int = 4) -> np.ndarray:
    sels = np.zeros((nsel, nsel, P), dtype=np.float32)
    for k in range(nsel):
        sels[k, k, :] = 1.0
    return sels


def kernel(bf: np.ndarray, aspp_mask: np.ndarray, min_std: np.ndarray, **run_kwargs):
    bf = np.asarray(bf, dtype=np.float32)
    aspp_mask = np.asarray(aspp_mask, dtype=np.float32)
    min_std = np.asarray(min_std, dtype=np.float32)
    B, C, H, W = bf.shape
    npix = H * W

    nc = _get_nc(npix)
    in_maps = make_in_maps(bf, aspp_mask, min_std)
    res = run_bass_kernel_spmd(nc, in_maps, list(range(N_CORES)), **run_kwargs)

    out = np.empty((B, C), dtype=np.float32)
    rows_per_core = (B * C) // N_CORES
    cores_per_batch = C // rows_per_core
    for k in range(N_CORES):
        b = k // cores_per_batch
        c0 = (k % cores_per_batch) * rows_per_core
        out[b, c0 : c0 + rows_per_core] = res.results[k]["out"].reshape(rows_per_core)
    if run_kwargs:
        return out.reshape(B, C, 1, 1), res
    return out.reshape(B, C, 1, 1)


# revision 42
# speedup vs baseline: 1.7401x; 1.7401x over previous
"""Trainium2 Bass kernel for nn_BackgroundStd2D.

Computes, per (batch, channel): the unbiased std of bf over the pixels where
aspp_mask <= 0.5, clamped below by min_std + 1e-6.

Sharding: pure data parallel over the 1024 (batch, channel) rows of
bf.reshape(B*C, H*W); each of the 8 cores handles 128 rows (one batch's
half of channels) plus that batch's mask.

Per-core algorithm (rows on partitions, pixels on the free axis):
  keep128[p, f] = (mask[p*512+f] <= 0.5) in bf16 (exact 0/1)  [128, 512]
  keep is re-laid out to [4, n_chunks, 512] via a DRAM bounce; per 2048-px
  chunk the PE broadcasts the 4 keep rows across all 128 partitions into
  PSUM with one-hot bf16 selector matmuls (bf16 keeps PE at full rate).
  DVE scalar_tensor_tensor (stock ISA): bfk = (bf*1)*keep written in place
  over the bf tile, accum_out = s_part (fused multiply+sum, one pass).
  ACT activation(Square, accum_out): q_part = sum(bfk^2), second pass.
  n = sum(keep) via free-axis reduce + DRAM-bounce partition fold.
  Final [128,1] math: std = sqrt((q - s^2/n) / (n-1)) (+ 2 Newton steps),
  out = max(std, min_std + 1e-6).
  Steady state is HBM-bound: ~100.5us/pass vs ~95.4us pure-DMA floor
  (32 MiB/core at ~352 GB/s).
"""

import sys

sys.path.insert(0, "/opt/trn_rl_repo")

import numpy as np

import concourse.bass as bass
import concourse.tile as tile
from concourse import bacc, mybir
from concourse.bass_utils import run_bass_kernel_spmd

P = 128
N_CORES = 8
MIN_STD_VAL = 1e-05

F32 = mybir.dt.float32
BF16 = mybir.dt.bfloat16
ALU = mybir.AluOpType
ACTF = mybir.ActivationFunctionType


def build_bass(
    npix: int,
    dma_chunk: int = 4096,
    ttr_chunk: int = 2048,
    iters: int = 1,
    mode: str = "full",  # full | noact | nostt | dmaonly
    bf_bufs: int = 3,
    hw_loop: bool = False,
    dual_ring: bool = False,
    in_place: bool = False,
    pixmaj: bool = False,
) -> bass.Bass:
    assert npix % dma_chunk == 0 and dma_chunk % ttr_chunk == 0
    assert ttr_chunk % 512 == 0
    n_blk = npix // 512  # 512-pixel blocks; one keep row per block
    assert n_blk <= P
    n_dma = npix // dma_chunk
    n_ttr = npix // ttr_chunk

    nc = bacc.Bacc("TRN2", target_bir_lowering=False, debug=False)

    if pixmaj:
        bf_d = nc.dram_tensor(
            "bf", [npix // dma_chunk, P, dma_chunk], F32, kind="ExternalInput"
        ).ap()
    else:
        bf_d = nc.dram_tensor("bf", [P, npix], F32, kind="ExternalInput").ap()
    mask_d = nc.dram_tensor("mask", [n_blk, 512], F32, kind="ExternalInput").ap()
    mins_d = nc.dram_tensor("min_std", [P, 1], F32, kind="ExternalInput").ap()
    out_d = nc.dram_tensor("out", [P, 1], F32, kind="ExternalOutput").ap()
    keep_scratch = nc.dram_tensor("keep_scratch", [npix], BF16).ap()
    cnt_scratch = nc.dram_tensor("cnt_scratch", [P], F32).ap()
    n_scratch = nc.dram_tensor("n_scratch", [1], F32).ap()
    nsel = ttr_chunk // 512
    sel_d = nc.dram_tensor("sels", [nsel, nsel, P], BF16, kind="ExternalInput").ap()

    with tile.TileContext(nc) as tc:
        with (
            tc.tile_pool(name="singles", bufs=1) as singles,
            tc.tile_pool(name="bfp", bufs=bf_bufs) as bf_pool,
            tc.tile_pool(name="bfkp", bufs=bf_bufs) as bfk_pool,
            tc.tile_pool(name="kps", bufs=2, space="PSUM") as kp_pool,
            tc.tile_pool(name="fin", bufs=2) as fin,
        ):
            # One-hot row selectors: sel[k].T @ keep_r[:, J, :] broadcasts
            # keep row k across all 128 output partitions.
            sel_t = singles.tile([nsel, nsel, P], BF16)
            nc.scalar.dma_start(out=sel_t, in_=sel_d)
            sels = [sel_t[:, k, :] for k in range(nsel)]

            mask128 = singles.tile([n_blk, 512], F32)
            nc.scalar.dma_start(out=mask128, in_=mask_d)
            # keep is exactly 0/1 so bf16 is lossless; bf16 operands keep the
            # PE broadcast matmuls at full (non-fp32) rate.
            keep128 = singles.tile([n_blk, 512], BF16)
            nc.vector.tensor_scalar(
                out=keep128, in0=mask128, scalar1=0.5, scalar2=None, op0=ALU.is_le
            )
            # Bounce through DRAM to land keep in [4, n_ttr, 512] layout:
            # partition a holds pixel blocks {4m + a}.
            nc.scalar.dma_start(out=keep_scratch, in_=keep128)
            keep_r = singles.tile([nsel, n_ttr, 512], BF16)
            nc.scalar.dma_start(
                out=keep_r,
                in_=keep_scratch.rearrange("(m a f) -> a m f", a=nsel, f=512),
            )

            # n = sum(keep): free-axis reduce, then fold the 128 partition
            # partials onto one partition via a DRAM bounce, reduce, and
            # broadcast the scalar back to all partitions.
            cnt = singles.tile([P, 1], F32)
            nc.vector.memset(cnt, 0.0)
            nc.vector.reduce_sum(
                out=cnt[0:n_blk, :], in_=keep128, axis=mybir.AxisListType.X
            )
            nc.scalar.dma_start(out=cnt_scratch, in_=cnt)
            cnt_row = singles.tile([1, P], F32)
            nc.scalar.dma_start(out=cnt_row, in_=cnt_scratch)
            n_scalar = singles.tile([1, 1], F32)
            nc.vector.reduce_sum(out=n_scalar, in_=cnt_row, axis=mybir.AxisListType.X)
            nc.scalar.dma_start(out=n_scratch, in_=n_scalar)
            n_b = singles.tile([P, 1], F32)
            nc.scalar.dma_start(out=n_b, in_=n_scratch.to_broadcast([P, 1]))

            minstd_sb = singles.tile([P, 1], F32)
            nc.scalar.dma_start(out=minstd_sb, in_=mins_d)

            s_parts = singles.tile([P, n_ttr], F32)
            q_parts = singles.tile([P, n_dma], F32)
            if mode != "full":
                nc.vector.memset(q_parts, 1.0)
                nc.vector.memset(s_parts, 1.0)

            import contextlib

            loop_cm = (
                tc.For_i(0, iters, 1) if hw_loop else contextlib.nullcontext(range(iters))
            )
            with loop_cm as _loop:
              for _it in range(1 if hw_loop else iters):
               for c in range(n_dma):
                bf_t = bf_pool.tile([P, dma_chunk], F32)
                dma_eng = nc.scalar if (dual_ring and c % 2) else nc.sync
                bf_src = bf_d[c] if pixmaj else bf_d[:, c * dma_chunk : (c + 1) * dma_chunk]
                dma_eng.dma_start(out=bf_t, in_=bf_src)
                bfk_t = bf_t if in_place else bfk_pool.tile([P, dma_chunk], F32)
                if mode == "dmaonly":
                    nc.vector.reduce_sum(
                        out=s_parts[:, c : c + 1],
                        in_=bf_t[:, 0:8],
                        axis=mybir.AxisListType.X,
                    )
                    continue
                for h in range(dma_chunk // ttr_chunk):
                    j = c * (dma_chunk // ttr_chunk) + h
                    kp = kp_pool.tile([P, ttr_chunk], F32)
                    for k in range(ttr_chunk // 512):
                        nc.tensor.matmul(
                            kp[:, 512 * k : 512 * (k + 1)],
                            sels[k],
                            keep_r[:, j, :],
                            start=True,
                            stop=True,
                        )
                    if mode != "nostt":
                        nc.vector.scalar_tensor_tensor(
                            out=bfk_t[:, h * ttr_chunk : (h + 1) * ttr_chunk],
                            in0=bf_t[:, h * ttr_chunk : (h + 1) * ttr_chunk],
                            scalar=1.0,
                            in1=kp,
                            op0=ALU.mult,
                            op1=ALU.mult,
                            accum_out=s_parts[:, j : j + 1],
                        )
                    else:
                        nc.vector.reduce_sum(
                            out=s_parts[:, j : j + 1],
                            in_=kp[:, 0:8],
                            axis=mybir.AxisListType.X,
                        )
                if mode == "full":
                    nc.scalar.activation(
                        out=bfk_t,
                        in_=bfk_t,
                        func=ACTF.Square,
                        accum_out=q_parts[:, c : c + 1],
                    )

            s = fin.tile([P, 1], F32)
            nc.vector.reduce_sum(out=s, in_=s_parts, axis=mybir.AxisListType.X)
            q = fin.tile([P, 1], F32)
            nc.vector.reduce_sum(out=q, in_=q_parts, axis=mybir.AxisListType.X)

            inv_n = fin.tile([P, 1], F32)
            nc.vector.reciprocal(inv_n, n_b)
            mean = fin.tile([P, 1], F32)
            nc.vector.tensor_mul(mean, s, inv_n)
            s2n = fin.tile([P, 1], F32)
            nc.vector.tensor_mul(s2n, mean, s)
            num = fin.tile([P, 1], F32)
            nc.vector.tensor_sub(num, q, s2n)
            nm1 = fin.tile([P, 1], F32)
            nc.vector.tensor_scalar_add(nm1, n_b, -1.0)
            inv_nm1 = fin.tile([P, 1], F32)
            nc.vector.reciprocal(inv_nm1, nm1)
            var = fin.tile([P, 1], F32)
            nc.vector.tensor_mul(var, num, inv_nm1)

            std = fin.tile([P, 1], F32)
            nc.scalar.sqrt(std, var)
            # ACT sqrt has a loose ULP budget; two Newton steps pin it to f32.
            for it in range(2):
                r = fin.tile([P, 1], F32, name=f"r{it}")
                nc.vector.reciprocal(r, std)
                t = fin.tile([P, 1], F32, name=f"t{it}")
                nc.vector.tensor_mul(t, var, r)
                u = fin.tile([P, 1], F32, name=f"u{it}")
                nc.vector.tensor_add(u, std, t)
                std = fin.tile([P, 1], F32, name=f"std{it}")
                nc.vector.tensor_scalar_mul(std, u, 0.5)

            lower = fin.tile([P, 1], F32)
            nc.vector.tensor_scalar_add(lower, minstd_sb, MIN_STD_VAL / 10.0)
            outv = fin.tile([P, 1], F32)
            nc.vector.tensor_max(outv, std, lower)
            nc.sync.dma_start(out=out_d, in_=outv)

    nc.compile()
    return nc


_NC_CACHE: dict[tuple, bass.Bass] = {}


def _get_nc(npix: int, **kwargs) -> bass.Bass:
    key = (npix, tuple(sorted(kwargs.items())))
    if key not in _NC_CACHE:
        _NC_CACHE[key] = build_bass(npix, **kwargs)
    return _NC_CACHE[key]


def make_in_maps(
    bf: np.ndarray,
    aspp_mask: np.ndarray,
    min_std: np.ndarray,
    pixmaj: bool = False,
    dma_chunk: int = 4096,
):
    B, C, H, W = bf.shape
    npix = H * W
    rows = bf.reshape(B * C, npix)
    rows_per_core = (B * C) // N_CORES
    cores_per_batch = C // rows_per_core
    mask_flat = np.ascontiguousarray(aspp_mask.reshape(B, npix))
    minstd_flat = np.ascontiguousarray(min_std.reshape(C))
    sels = make_sels()
    in_maps = []
    for k in range(N_CORES):
        b = k // cores_per_batch
        c0 = (k % cores_per_batch) * rows_per_core
        shard = rows[k * rows_per_core : (k + 1) * rows_per_core]
        if pixmaj:
            # [n_dma, P, dma_chunk]: each chunk contiguous in DRAM
            shard = np.ascontiguousarray(
                shard.reshape(P, npix // dma_chunk, dma_chunk).transpose(1, 0, 2)
            )
        else:
            shard = np.ascontiguousarray(shard)
        in_maps.append(
            {
                "bf": shard,
                "mask": mask_flat[b].reshape(npix // 512, 512),
                "min_std": minstd_flat[c0 : c0 + rows_per_core].reshape(P, 1),
                "sels": sels,
            }
        )
    return in_maps


def make_sels(nsel: int = 4) -> np.ndarray:
    import ml_dtypes

    sels = np.zeros((nsel, nsel, P), dtype=ml_dtypes.bfloat16)
    for k in range(nsel):
        sels[k, k, :] = 1.0
    return sels


def kernel(bf: np.ndarray, aspp_mask: np.ndarray, min_std: np.ndarray, **run_kwargs):
    bf = np.asarray(bf, dtype=np.float32)
    aspp_mask = np.asarray(aspp_mask, dtype=np.float32)
    min_std = np.asarray(min_std, dtype=np.float32)
    B, C, H, W = bf.shape
    npix = H * W

    nc = _get_nc(npix, dma_chunk=2048, bf_bufs=12, in_place=True)
    in_maps = make_in_maps(bf, aspp_mask, min_std)
    res = run_bass_kernel_spmd(nc, in_maps, list(range(N_CORES)), **run_kwargs)

    out = np.empty((B, C), dtype=np.float32)
    rows_per_core = (B * C) // N_CORES
    cores_per_batch = C // rows_per_core
    for k in range(N_CORES):
        b = k // cores_per_batch
        c0 = (k % cores_per_batch) * rows_per_core
        out[b, c0 : c0 + rows_per_core] = res.results[k]["out"].reshape(rows_per_core)
    if run_kwargs:
        return out.reshape(B, C, 1, 1), res
    return out.reshape(B, C, 1, 1)


# revision 43
# speedup vs baseline: 1.7589x; 1.0108x over previous
"""Trainium2 Bass kernel for nn_BackgroundStd2D.

Computes, per (batch, channel): the unbiased std of bf over the pixels where
aspp_mask <= 0.5, clamped below by min_std + 1e-6.

Sharding: pure data parallel over the 1024 (batch, channel) rows of
bf.reshape(B*C, H*W); each of the 8 cores handles 128 rows (one batch's
half of channels) plus that batch's mask.

Per-core algorithm (rows on partitions, pixels on the free axis):
  keep128[p, f] = (mask[p*512+f] <= 0.5) in bf16 (exact 0/1)  [128, 512]
  keep is re-laid out to [4, n_chunks, 512] via a DRAM bounce; per 2048-px
  chunk the PE broadcasts the 4 keep rows across all 128 partitions into
  PSUM with one-hot bf16 selector matmuls (bf16 keeps PE at full rate).
  DVE scalar_tensor_tensor (stock ISA): bfk = (bf*1)*keep written in place
  over the bf tile, accum_out = s_part (fused multiply+sum, one pass).
  ACT activation(Square, accum_out): q_part = sum(bfk^2), second pass.
  n = sum(keep) via free-axis reduce + DRAM-bounce partition fold.
  Final [128,1] math: std = sqrt((q - s^2/n) / (n-1)) (+ 2 Newton steps),
  out = max(std, min_std + 1e-6).
  Steady state is HBM-bound: ~100.5us/pass vs ~95.4us pure-DMA floor
  (32 MiB/core at ~352 GB/s).
"""

import sys

sys.path.insert(0, "/opt/trn_rl_repo")

import numpy as np

import concourse.bass as bass
import concourse.tile as tile
from concourse import bacc, mybir
from concourse.bass_utils import run_bass_kernel_spmd

P = 128
N_CORES = 8
MIN_STD_VAL = 1e-05

F32 = mybir.dt.float32
BF16 = mybir.dt.bfloat16
ALU = mybir.AluOpType
ACTF = mybir.ActivationFunctionType


def build_bass(
    npix: int,
    dma_chunk: int = 4096,
    ttr_chunk: int = 2048,
    iters: int = 1,
    mode: str = "full",  # full | noact | nostt | dmaonly
    bf_bufs: int = 3,
    hw_loop: bool = False,
    dual_ring: bool = False,
    in_place: bool = False,
    pixmaj: bool = False,
) -> bass.Bass:
    assert npix % dma_chunk == 0 and dma_chunk % ttr_chunk == 0
    assert ttr_chunk % 512 == 0
    n_blk = npix // 512  # 512-pixel blocks; one keep row per block
    assert n_blk <= P
    n_dma = npix // dma_chunk
    n_ttr = npix // ttr_chunk

    nc = bacc.Bacc("TRN2", target_bir_lowering=False, debug=False)

    if pixmaj:
        bf_d = nc.dram_tensor(
            "bf", [npix // dma_chunk, P, dma_chunk], F32, kind="ExternalInput"
        ).ap()
    else:
        bf_d = nc.dram_tensor("bf", [P, npix], F32, kind="ExternalInput").ap()
    mask_d = nc.dram_tensor("mask", [n_blk, 512], F32, kind="ExternalInput").ap()
    mins_d = nc.dram_tensor("min_std", [P, 1], F32, kind="ExternalInput").ap()
    out_d = nc.dram_tensor("out", [P, 1], F32, kind="ExternalOutput").ap()
    keep_scratch = nc.dram_tensor("keep_scratch", [npix], BF16).ap()
    cnt_scratch = nc.dram_tensor("cnt_scratch", [P], F32).ap()
    n_scratch = nc.dram_tensor("n_scratch", [1], F32).ap()
    nsel = ttr_chunk // 512
    sel_d = nc.dram_tensor("sels", [nsel, nsel, P], BF16, kind="ExternalInput").ap()

    with tile.TileContext(nc) as tc:
        with (
            tc.tile_pool(name="singles", bufs=1) as singles,
            tc.tile_pool(name="bfp", bufs=bf_bufs) as bf_pool,
            tc.tile_pool(name="bfkp", bufs=bf_bufs) as bfk_pool,
            tc.tile_pool(name="kps", bufs=2, space="PSUM") as kp_pool,
            tc.tile_pool(name="fin", bufs=2) as fin,
        ):
            # One-hot row selectors: sel[k].T @ keep_r[:, J, :] broadcasts
            # keep row k across all 128 output partitions.
            sel_t = singles.tile([nsel, nsel, P], BF16)
            nc.scalar.dma_start(out=sel_t, in_=sel_d)
            sels = [sel_t[:, k, :] for k in range(nsel)]

            mask128 = singles.tile([n_blk, 512], F32)
            nc.scalar.dma_start(out=mask128, in_=mask_d)
            # keep is exactly 0/1 so bf16 is lossless; bf16 operands keep the
            # PE broadcast matmuls at full (non-fp32) rate.
            keep128 = singles.tile([n_blk, 512], BF16)
            nc.vector.tensor_scalar(
                out=keep128, in0=mask128, scalar1=0.5, scalar2=None, op0=ALU.is_le
            )
            # Bounce through DRAM to land keep in [4, n_ttr, 512] layout:
            # partition a holds pixel blocks {4m + a}.
            nc.scalar.dma_start(out=keep_scratch, in_=keep128)
            keep_r = singles.tile([nsel, n_ttr, 512], BF16)
            nc.scalar.dma_start(
                out=keep_r,
                in_=keep_scratch.rearrange("(m a f) -> a m f", a=nsel, f=512),
            )

            # n = sum(keep): free-axis reduce, then fold the 128 partition
            # partials onto one partition via a DRAM bounce, reduce, and
            # broadcast the scalar back to all partitions.
            cnt = singles.tile([P, 1], F32)
            nc.vector.memset(cnt, 0.0)
            nc.vector.reduce_sum(
                out=cnt[0:n_blk, :], in_=keep128, axis=mybir.AxisListType.X
            )
            nc.scalar.dma_start(out=cnt_scratch, in_=cnt)
            cnt_row = singles.tile([1, P], F32)
            nc.scalar.dma_start(out=cnt_row, in_=cnt_scratch)
            n_scalar = singles.tile([1, 1], F32)
            nc.vector.reduce_sum(out=n_scalar, in_=cnt_row, axis=mybir.AxisListType.X)
            nc.scalar.dma_start(out=n_scratch, in_=n_scalar)
            n_b = singles.tile([P, 1], F32)
            nc.scalar.dma_start(out=n_b, in_=n_scratch.to_broadcast([P, 1]))

            minstd_sb = singles.tile([P, 1], F32)
            nc.scalar.dma_start(out=minstd_sb, in_=mins_d)

            s_parts = singles.tile([P, n_ttr], F32)
            q_parts = singles.tile([P, n_dma], F32)
            if mode != "full":
                nc.vector.memset(q_parts, 1.0)
                nc.vector.memset(s_parts, 1.0)

            import contextlib

            loop_cm = (
                tc.For_i(0, iters, 1) if hw_loop else contextlib.nullcontext(range(iters))
            )
            with loop_cm as _loop:
              for _it in range(1 if hw_loop else iters):
               for c in range(n_dma):
                bf_t = bf_pool.tile([P, dma_chunk], F32)
                dma_eng = nc.scalar if (dual_ring and c % 2) else nc.sync
                bf_src = bf_d[c] if pixmaj else bf_d[:, c * dma_chunk : (c + 1) * dma_chunk]
                dma_eng.dma_start(out=bf_t, in_=bf_src)
                bfk_t = bf_t if in_place else bfk_pool.tile([P, dma_chunk], F32)
                if mode == "dmaonly":
                    nc.vector.reduce_sum(
                        out=s_parts[:, c : c + 1],
                        in_=bf_t[:, 0:8],
                        axis=mybir.AxisListType.X,
                    )
                    continue
                for h in range(dma_chunk // ttr_chunk):
                    j = c * (dma_chunk // ttr_chunk) + h
                    kp = kp_pool.tile([P, ttr_chunk], F32)
                    for k in range(ttr_chunk // 512):
                        nc.tensor.matmul(
                            kp[:, 512 * k : 512 * (k + 1)],
                            sels[k],
                            keep_r[:, j, :],
                            start=True,
                            stop=True,
                        )
                    if mode != "nostt":
                        nc.vector.scalar_tensor_tensor(
                            out=bfk_t[:, h * ttr_chunk : (h + 1) * ttr_chunk],
                            in0=bf_t[:, h * ttr_chunk : (h + 1) * ttr_chunk],
                            scalar=1.0,
                            in1=kp,
                            op0=ALU.mult,
                            op1=ALU.mult,
                            accum_out=s_parts[:, j : j + 1],
                        )
                    else:
                        nc.vector.reduce_sum(
                            out=s_parts[:, j : j + 1],
                            in_=kp[:, 0:8],
                            axis=mybir.AxisListType.X,
                        )
                if mode == "full":
                    nc.scalar.activation(
                        out=bfk_t,
                        in_=bfk_t,
                        func=ACTF.Square,
                        accum_out=q_parts[:, c : c + 1],
                    )

            s = fin.tile([P, 1], F32)
            nc.vector.reduce_sum(out=s, in_=s_parts, axis=mybir.AxisListType.X)
            q = fin.tile([P, 1], F32)
            nc.vector.reduce_sum(out=q, in_=q_parts, axis=mybir.AxisListType.X)

            inv_n = fin.tile([P, 1], F32)
            nc.vector.reciprocal(inv_n, n_b)
            mean = fin.tile([P, 1], F32)
            nc.vector.tensor_mul(mean, s, inv_n)
            s2n = fin.tile([P, 1], F32)
            nc.vector.tensor_mul(s2n, mean, s)
            num = fin.tile([P, 1], F32)
            nc.vector.tensor_sub(num, q, s2n)
            nm1 = fin.tile([P, 1], F32)
            nc.vector.tensor_scalar_add(nm1, n_b, -1.0)
            inv_nm1 = fin.tile([P, 1], F32)
            nc.vector.reciprocal(inv_nm1, nm1)
            var = fin.tile([P, 1], F32)
            nc.vector.tensor_mul(var, num, inv_nm1)

            std = fin.tile([P, 1], F32)
            nc.scalar.sqrt(std, var)
            # ACT sqrt has a loose ULP budget; two Newton steps pin it to f32.
            for it in range(2):
                r = fin.tile([P, 1], F32, name=f"r{it}")
                nc.vector.reciprocal(r, std)
                t = fin.tile([P, 1], F32, name=f"t{it}")
                nc.vector.tensor_mul(t, var, r)
                u = fin.tile([P, 1], F32, name=f"u{it}")
                nc.vector.tensor_add(u, std, t)
                std = fin.tile([P, 1], F32, name=f"std{it}")
                nc.vector.tensor_scalar_mul(std, u, 0.5)

            lower = fin.tile([P, 1], F32)
            nc.vector.tensor_scalar_add(lower, minstd_sb, MIN_STD_VAL / 10.0)
            outv = fin.tile([P, 1], F32)
            nc.vector.tensor_max(outv, std, lower)
            nc.sync.dma_start(out=out_d, in_=outv)

    nc.compile()
    return nc


_NC_CACHE: dict[tuple, bass.Bass] = {}


def _get_nc(npix: int, **kwargs) -> bass.Bass:
    key = (npix, tuple(sorted(kwargs.items())))
    if key not in _NC_CACHE:
        _NC_CACHE[key] = build_bass(npix, **kwargs)
    return _NC_CACHE[key]


def make_in_maps(
    bf: np.ndarray,
    aspp_mask: np.ndarray,
    min_std: np.ndarray,
    pixmaj: bool = False,
    dma_chunk: int = 4096,
    ttr_chunk: int = 2048,
):
    B, C, H, W = bf.shape
    npix = H * W
    rows = bf.reshape(B * C, npix)
    rows_per_core = (B * C) // N_CORES
    cores_per_batch = C // rows_per_core
    mask_flat = np.ascontiguousarray(aspp_mask.reshape(B, npix))
    minstd_flat = np.ascontiguousarray(min_std.reshape(C))
    sels = make_sels(ttr_chunk // 512)
    in_maps = []
    for k in range(N_CORES):
        b = k // cores_per_batch
        c0 = (k % cores_per_batch) * rows_per_core
        shard = rows[k * rows_per_core : (k + 1) * rows_per_core]
        if pixmaj:
            # [n_dma, P, dma_chunk]: each chunk contiguous in DRAM
            shard = np.ascontiguousarray(
                shard.reshape(P, npix // dma_chunk, dma_chunk).transpose(1, 0, 2)
            )
        else:
            shard = np.ascontiguousarray(shard)
        in_maps.append(
            {
                "bf": shard,
                "mask": mask_flat[b].reshape(npix // 512, 512),
                "min_std": minstd_flat[c0 : c0 + rows_per_core].reshape(P, 1),
                "sels": sels,
            }
        )
    return in_maps


def make_sels(nsel: int = 4) -> np.ndarray:
    import ml_dtypes

    sels = np.zeros((nsel, nsel, P), dtype=ml_dtypes.bfloat16)
    for k in range(nsel):
        sels[k, k, :] = 1.0
    return sels


def kernel(bf: np.ndarray, aspp_mask: np.ndarray, min_std: np.ndarray, **run_kwargs):
    bf = np.asarray(bf, dtype=np.float32)
    aspp_mask = np.asarray(aspp_mask, dtype=np.float32)
    min_std = np.asarray(min_std, dtype=np.float32)
    B, C, H, W = bf.shape
    npix = H * W

    nc = _get_nc(npix, dma_chunk=2048, bf_bufs=12, in_place=True)
    in_maps = make_in_maps(bf, aspp_mask, min_std)
    res = run_bass_kernel_spmd(nc, in_maps, list(range(N_CORES)), **run_kwargs)

    out = np.empty((B, C), dtype=np.float32)
    rows_per_core = (B * C) // N_CORES
    cores_per_batch = C // rows_per_core
    for k in range(N_CORES):
        b = k // cores_per_batch
        c0 = (k % cores_per_batch) * rows_per_core
        out[b, c0 : c0 + rows_per_core] = res.results[k]["out"].reshape(rows_per_core)
    if run_kwargs:
        return out.reshape(B, C, 1, 1), res
    return out.reshape(B, C, 1, 1)


# revision 44
# speedup vs baseline: 1.7958x; 1.0210x over previous
"""Trainium2 Bass kernel for nn_BackgroundStd2D.

Computes, per (batch, channel): the unbiased std of bf over the pixels where
aspp_mask <= 0.5, clamped below by min_std + 1e-6.

Sharding: pure data parallel over the 1024 (batch, channel) rows of
bf.reshape(B*C, H*W); each of the 8 cores handles 128 rows (one batch's
half of channels) plus that batch's mask.

Per-core algorithm (rows on partitions, pixels on the free axis):
  keep128[p, f] = (mask[p*512+f] <= 0.5) in bf16 (exact 0/1)  [128, 512]
  keep is re-laid out to [4, n_chunks, 512] via a DRAM bounce; per 2048-px
  chunk the PE broadcasts the 4 keep rows across all 128 partitions into
  PSUM with one-hot bf16 selector matmuls (bf16 keeps PE at full rate).
  DVE scalar_tensor_tensor (stock ISA): bfk = (bf*1)*keep written in place
  over the bf tile, accum_out = s_part (fused multiply+sum, one pass).
  ACT activation(Square, accum_out): q_part = sum(bfk^2), second pass.
  n = sum(keep) via free-axis reduce + DRAM-bounce partition fold.
  Final [128,1] math: std = sqrt((q - s^2/n) / (n-1)) (+ 2 Newton steps),
  out = max(std, min_std + 1e-6).
  Steady state is HBM-bound: ~100.5us/pass vs ~95.4us pure-DMA floor
  (32 MiB/core at ~352 GB/s).
"""

import sys

sys.path.insert(0, "/opt/trn_rl_repo")

import numpy as np

import concourse.bass as bass
import concourse.tile as tile
from concourse import bacc, mybir
from concourse.bass_utils import run_bass_kernel_spmd

P = 128
N_CORES = 8
MIN_STD_VAL = 1e-05

F32 = mybir.dt.float32
BF16 = mybir.dt.bfloat16
ALU = mybir.AluOpType
ACTF = mybir.ActivationFunctionType


def build_bass(
    npix: int,
    dma_chunk: int = 4096,
    ttr_chunk: int = 2048,
    iters: int = 1,
    mode: str = "full",  # full | noact | nostt | dmaonly
    bf_bufs: int = 3,
    hw_loop: bool = False,
    dual_ring: bool = False,
    in_place: bool = False,
    pixmaj: bool = False,
) -> bass.Bass:
    assert npix % dma_chunk == 0 and dma_chunk % ttr_chunk == 0
    assert ttr_chunk % 512 == 0
    n_blk = npix // 512  # 512-pixel blocks; one keep row per block
    assert n_blk <= P
    n_dma = npix // dma_chunk
    n_ttr = npix // ttr_chunk

    nc = bacc.Bacc("TRN2", target_bir_lowering=False, debug=False)

    if pixmaj:
        bf_d = nc.dram_tensor(
            "bf", [npix // dma_chunk, P, dma_chunk], F32, kind="ExternalInput"
        ).ap()
    else:
        bf_d = nc.dram_tensor("bf", [P, npix], F32, kind="ExternalInput").ap()
    mask_d = nc.dram_tensor("mask", [n_blk, 512], F32, kind="ExternalInput").ap()
    mins_d = nc.dram_tensor("min_std", [P, 1], F32, kind="ExternalInput").ap()
    out_d = nc.dram_tensor("out", [P, 1], F32, kind="ExternalOutput").ap()
    keep_scratch = nc.dram_tensor("keep_scratch", [npix], BF16).ap()
    cnt_scratch = nc.dram_tensor("cnt_scratch", [P], F32).ap()
    n_scratch = nc.dram_tensor("n_scratch", [1], F32).ap()
    nsel = ttr_chunk // 512
    sel_d = nc.dram_tensor("sels", [nsel, nsel, P], BF16, kind="ExternalInput").ap()

    with tile.TileContext(nc) as tc:
        with (
            tc.tile_pool(name="singles", bufs=1) as singles,
            tc.tile_pool(name="bfp", bufs=bf_bufs) as bf_pool,
            tc.tile_pool(name="bfkp", bufs=bf_bufs) as bfk_pool,
            tc.tile_pool(name="kps", bufs=2, space="PSUM") as kp_pool,
            tc.tile_pool(name="fin", bufs=2) as fin,
        ):
            # One-hot row selectors: sel[k].T @ keep_r[:, J, :] broadcasts
            # keep row k across all 128 output partitions.
            sel_t = singles.tile([nsel, nsel, P], BF16)
            nc.scalar.dma_start(out=sel_t, in_=sel_d)
            sels = [sel_t[:, k, :] for k in range(nsel)]

            mask128 = singles.tile([n_blk, 512], F32)
            nc.scalar.dma_start(out=mask128, in_=mask_d)
            # keep is exactly 0/1 so bf16 is lossless; bf16 operands keep the
            # PE broadcast matmuls at full (non-fp32) rate.
            keep128 = singles.tile([n_blk, 512], BF16)
            nc.vector.tensor_scalar(
                out=keep128, in0=mask128, scalar1=0.5, scalar2=None, op0=ALU.is_le
            )
            # Bounce through DRAM to land keep in [4, n_ttr, 512] layout:
            # partition a holds pixel blocks {4m + a}.
            nc.scalar.dma_start(out=keep_scratch, in_=keep128)
            keep_r = singles.tile([nsel, n_ttr, 512], BF16)
            nc.scalar.dma_start(
                out=keep_r,
                in_=keep_scratch.rearrange("(m a f) -> a m f", a=nsel, f=512),
            )

            # n = sum(keep): free-axis reduce, then fold the 128 partition
            # partials onto one partition via a DRAM bounce, reduce, and
            # broadcast the scalar back to all partitions.
            cnt = singles.tile([P, 1], F32)
            nc.vector.memset(cnt, 0.0)
            nc.vector.reduce_sum(
                out=cnt[0:n_blk, :], in_=keep128, axis=mybir.AxisListType.X
            )
            nc.scalar.dma_start(out=cnt_scratch, in_=cnt)
            cnt_row = singles.tile([1, P], F32)
            nc.scalar.dma_start(out=cnt_row, in_=cnt_scratch)
            n_scalar = singles.tile([1, 1], F32)
            nc.vector.reduce_sum(out=n_scalar, in_=cnt_row, axis=mybir.AxisListType.X)
            nc.scalar.dma_start(out=n_scratch, in_=n_scalar)
            n_b = singles.tile([P, 1], F32)
            nc.scalar.dma_start(out=n_b, in_=n_scratch.to_broadcast([P, 1]))

            minstd_sb = singles.tile([P, 1], F32)
            nc.scalar.dma_start(out=minstd_sb, in_=mins_d)

            s_parts = singles.tile([P, n_ttr], F32)
            q_parts = singles.tile([P, n_dma], F32)
            if mode != "full":
                nc.vector.memset(q_parts, 1.0)
                nc.vector.memset(s_parts, 1.0)

            import contextlib

            loop_cm = (
                tc.For_i(0, iters, 1) if hw_loop else contextlib.nullcontext(range(iters))
            )
            with loop_cm as _loop:
              for _it in range(1 if hw_loop else iters):
               for c in range(n_dma):
                bf_t = bf_pool.tile([P, dma_chunk], F32)
                dma_eng = nc.scalar if (dual_ring and c % 2) else nc.sync
                bf_src = bf_d[c] if pixmaj else bf_d[:, c * dma_chunk : (c + 1) * dma_chunk]
                dma_eng.dma_start(out=bf_t, in_=bf_src)
                bfk_t = bf_t if in_place else bfk_pool.tile([P, dma_chunk], F32)
                if mode == "dmaonly":
                    nc.vector.reduce_sum(
                        out=s_parts[:, c : c + 1],
                        in_=bf_t[:, 0:8],
                        axis=mybir.AxisListType.X,
                    )
                    continue
                for h in range(dma_chunk // ttr_chunk):
                    j = c * (dma_chunk // ttr_chunk) + h
                    kp = kp_pool.tile([P, ttr_chunk], F32)
                    for k in range(ttr_chunk // 512):
                        nc.tensor.matmul(
                            kp[:, 512 * k : 512 * (k + 1)],
                            sels[k],
                            keep_r[:, j, :],
                            start=True,
                            stop=True,
                        )
                    if mode != "nostt":
                        nc.vector.scalar_tensor_tensor(
                            out=bfk_t[:, h * ttr_chunk : (h + 1) * ttr_chunk],
                            in0=bf_t[:, h * ttr_chunk : (h + 1) * ttr_chunk],
                            scalar=1.0,
                            in1=kp,
                            op0=ALU.mult,
                            op1=ALU.mult,
                            accum_out=s_parts[:, j : j + 1],
                        )
                    else:
                        nc.vector.reduce_sum(
                            out=s_parts[:, j : j + 1],
                            in_=kp[:, 0:8],
                            axis=mybir.AxisListType.X,
                        )
                if mode == "full":
                    nc.scalar.activation(
                        out=bfk_t,
                        in_=bfk_t,
                        func=ACTF.Square,
                        accum_out=q_parts[:, c : c + 1],
                    )

            s = fin.tile([P, 1], F32)
            nc.vector.reduce_sum(out=s, in_=s_parts, axis=mybir.AxisListType.X)
            q = fin.tile([P, 1], F32)
            nc.vector.reduce_sum(out=q, in_=q_parts, axis=mybir.AxisListType.X)

            inv_n = fin.tile([P, 1], F32)
            nc.vector.reciprocal(inv_n, n_b)
            mean = fin.tile([P, 1], F32)
            nc.vector.tensor_mul(mean, s, inv_n)
            s2n = fin.tile([P, 1], F32)
            nc.vector.tensor_mul(s2n, mean, s)
            num = fin.tile([P, 1], F32)
            nc.vector.tensor_sub(num, q, s2n)
            nm1 = fin.tile([P, 1], F32)
            nc.vector.tensor_scalar_add(nm1, n_b, -1.0)
            inv_nm1 = fin.tile([P, 1], F32)
            nc.vector.reciprocal(inv_nm1, nm1)
            var = fin.tile([P, 1], F32)
            nc.vector.tensor_mul(var, num, inv_nm1)

            std = fin.tile([P, 1], F32)
            nc.scalar.sqrt(std, var)
            # ACT sqrt has a loose ULP budget; two Newton steps pin it to f32.
            for it in range(2):
                r = fin.tile([P, 1], F32, name=f"r{it}")
                nc.vector.reciprocal(r, std)
                t = fin.tile([P, 1], F32, name=f"t{it}")
                nc.vector.tensor_mul(t, var, r)
                u = fin.tile([P, 1], F32, name=f"u{it}")
                nc.vector.tensor_add(u, std, t)
                std = fin.tile([P, 1], F32, name=f"std{it}")
                nc.vector.tensor_scalar_mul(std, u, 0.5)

            lower = fin.tile([P, 1], F32)
            nc.vector.tensor_scalar_add(lower, minstd_sb, MIN_STD_VAL / 10.0)
            outv = fin.tile([P, 1], F32)
            nc.vector.tensor_max(outv, std, lower)
            nc.sync.dma_start(out=out_d, in_=outv)

    nc.compile()
    return nc


_NC_CACHE: dict[tuple, bass.Bass] = {}


def _get_nc(npix: int, **kwargs) -> bass.Bass:
    key = (npix, tuple(sorted(kwargs.items())))
    if key not in _NC_CACHE:
        _NC_CACHE[key] = build_bass(npix, **kwargs)
    return _NC_CACHE[key]


def make_in_maps(
    bf: np.ndarray,
    aspp_mask: np.ndarray,
    min_std: np.ndarray,
    pixmaj: bool = False,
    dma_chunk: int = 4096,
    ttr_chunk: int = 2048,
):
    B, C, H, W = bf.shape
    npix = H * W
    rows = bf.reshape(B * C, npix)
    rows_per_core = (B * C) // N_CORES
    cores_per_batch = C // rows_per_core
    mask_flat = np.ascontiguousarray(aspp_mask.reshape(B, npix))
    minstd_flat = np.ascontiguousarray(min_std.reshape(C))
    sels = make_sels(ttr_chunk // 512)
    in_maps = []
    for k in range(N_CORES):
        b = k // cores_per_batch
        c0 = (k % cores_per_batch) * rows_per_core
        shard = rows[k * rows_per_core : (k + 1) * rows_per_core]
        if pixmaj:
            # [n_dma, P, dma_chunk]: each chunk contiguous in DRAM
            shard = np.ascontiguousarray(
                shard.reshape(P, npix // dma_chunk, dma_chunk).transpose(1, 0, 2)
            )
        else:
            shard = np.ascontiguousarray(shard)
        in_maps.append(
            {
                "bf": shard,
                "mask": mask_flat[b].reshape(npix // 512, 512),
                "min_std": minstd_flat[c0 : c0 + rows_per_core].reshape(P, 1),
                "sels": sels,
            }
        )
    return in_maps


def make_sels(nsel: int = 4) -> np.ndarray:
    import ml_dtypes

    sels = np.zeros((nsel, nsel, P), dtype=ml_dtypes.bfloat16)
    for k in range(nsel):
        sels[k, k, :] = 1.0
    return sels


def kernel(bf: np.ndarray, aspp_mask: np.ndarray, min_std: np.ndarray, **run_kwargs):
    bf = np.asarray(bf, dtype=np.float32)
    aspp_mask = np.asarray(aspp_mask, dtype=np.float32)
    min_std = np.asarray(min_std, dtype=np.float32)
    B, C, H, W = bf.shape
    npix = H * W

    nc = _get_nc(npix, dma_chunk=2048, bf_bufs=16, in_place=True)
    in_maps = make_in_maps(bf, aspp_mask, min_std)
    res = run_bass_kernel_spmd(nc, in_maps, list(range(N_CORES)), **run_kwargs)

    out = np.empty((B, C), dtype=np.float32)
    rows_per_core = (B * C) // N_CORES
    cores_per_batch = C // rows_per_core
    for k in range(N_CORES):
        b = k // cores_per_batch
        c0 = (k % cores_per_batch) * rows_per_core
        out[b, c0 : c0 + rows_per_core] = res.results[k]["out"].reshape(rows_per_core)
    if run_kwargs:
        return out.reshape(B, C, 1, 1), res
    return out.reshape(B, C, 1, 1)
